# revision 1
# baseline (speedup 1.0000x reference)
"""Causal depthwise temporal conv (K=4) on 8 TRN2 NeuronCores.

Reference semantics (for x: [B, T, D], w: [K, D], b: [D]):
    out[bt, t, d] = sum_{j=0}^{K-1} x_pad[bt, t + j, d] * w[j, d] + b[d]
where x_pad is x left-padded with K-1 zeros along time.

Strategy:
  - Tensor-parallel over the channel axis: core m owns channels
    [m*512, (m+1)*512) -- the conv is depthwise so channels are fully
    independent (no collectives).
  - Host pre-transposes each core's shard to channel-major [D_sh, B, T+K-1]
    (left zero-padded). On device, channels sit on SBUF partitions so the
    per-channel weight becomes a per-partition scalar, and the temporal
    shifts become free-dimension slices.
  - Per (channel-block, batch): one ACT op computes w0*x + b, then three
    DVE scalar_tensor_tensor ops each fuse (x_shift * w_j) + acc.
    All DMAs are large contiguous HWDGE transfers.
"""

import numpy as np

import concourse.bacc as bacc
import concourse.mybir as mybir
from concourse.tile import TileContext
from concourse import bass_utils

B = 4            # batch
T = 4096         # sequence length
D = 4096         # channels (width)
K = 4            # temporal taps
N_CORES = 8
D_SH = D // N_CORES          # 512 channels per core
P = 128                      # SBUF partitions
N_BLK = D_SH // P            # 4 channel blocks per core
TP = T + K - 1               # padded time length


def _build(b=B, t=T, n_blk=N_BLK, batch_pair=2):
    nc = bacc.Bacc("TRN2")
    tp = t + K - 1
    f32 = mybir.dt.float32
    x = nc.dram_tensor("x", [n_blk, P, b, tp], f32, kind="ExternalInput")
    wb = nc.dram_tensor("wb", [n_blk, P, K + 1], f32, kind="ExternalInput")
    out = nc.dram_tensor("out", [n_blk, P, b, t], f32, kind="ExternalOutput")
    mult, add = mybir.AluOpType.mult, mybir.AluOpType.add
    ident_fn = mybir.ActivationFunctionType.Identity

    with TileContext(nc) as tc:
        with tc.tile_pool(name="pool", bufs=4) as pool, \
             tc.tile_pool(name="poola", bufs=3) as poola:
            for blk in range(n_blk):
                wt = pool.tile([P, K + 1], f32, tag="wb")
                nc.sync.dma_start(wt[:], wb[blk])
                for bb in range(b):
                    # The first and last chains run at half width so the
                    # DVE stream starts earlier (smaller first load+ACT)
                    # and the final store drains sooner.
                    edge = (blk == 0 and bb == 0) or \
                           (blk == n_blk - 1 and bb == b - 1)
                    if edge and t % 2 == 0 and t >= 2048:
                        hw_ = t // 2
                        for h in (0, hw_):
                            Xh = pool.tile([P, hw_ + K - 1], f32, tag="x")
                            nc.sync.dma_start(
                                Xh[:], x[blk, :, bb, h:h + hw_ + K - 1])
                            a0 = poola.tile([P, hw_], f32, tag="accA")
                            nc.scalar.activation(a0[:], Xh[:, 0:hw_],
                                                 ident_fn,
                                                 bias=wt[:, K:K + 1],
                                                 scale=wt[:, 0:1])
                            a1 = poola.tile([P, hw_], f32, tag="accB")
                            nc.vector.scalar_tensor_tensor(
                                a1[:], Xh[:, 1:1 + hw_], wt[:, 1:2], a0[:],
                                mult, add)
                            a2 = poola.tile([P, hw_], f32, tag="accA")
                            nc.vector.scalar_tensor_tensor(
                                a2[:], Xh[:, 2:2 + hw_], wt[:, 2:3], a1[:],
                                mult, add)
                            a3 = poola.tile([P, hw_], f32, tag="accB")
                            nc.vector.scalar_tensor_tensor(
                                a3[:], Xh[:, 3:3 + hw_], wt[:, 3:4], a2[:],
                                mult, add)
                            nc.sync.dma_start(
                                out[blk, :, bb, h:h + hw_], a3[:])
                        continue
                    # Per-batch loads (2.1MB) shorten the pipeline ramp;
                    # bufs=4 keeps several loads in flight.
                    X = pool.tile([P, tp], f32, tag="x")
                    nc.sync.dma_start(X[:], x[blk, :, bb, :])
                    # Per-batch chain, ping-pong accumulators:
                    # ACT does w0*x0+b, DVE does 3 fused FMAs.
                    a0 = poola.tile([P, t], f32, tag="accA")
                    nc.scalar.activation(a0[:], X[:, 0:t], ident_fn,
                                         bias=wt[:, K:K + 1],
                                         scale=wt[:, 0:1])
                    a1 = poola.tile([P, t], f32, tag="accB")
                    nc.vector.scalar_tensor_tensor(
                        a1[:], X[:, 1:1 + t], wt[:, 1:2], a0[:],
                        mult, add)
                    a2 = poola.tile([P, t], f32, tag="accA")
                    nc.vector.scalar_tensor_tensor(
                        a2[:], X[:, 2:2 + t], wt[:, 2:3], a1[:],
                        mult, add)
                    a3 = poola.tile([P, t], f32, tag="accB")
                    nc.vector.scalar_tensor_tensor(
                        a3[:], X[:, 3:3 + t], wt[:, 3:4], a2[:],
                        mult, add)
                    nc.sync.dma_start(out[blk, :, bb, :], a3[:])
    nc.compile()
    return nc


def _prepare(x, w, b):
    x = np.asarray(x, dtype=np.float32)
    w = np.asarray(w, dtype=np.float32)
    b = np.asarray(b, dtype=np.float32)
    # channel-major, left zero-padded time: [D, B, TP]
    xp = np.zeros((D, B, TP), dtype=np.float32)
    xp[:, :, K - 1:] = x.transpose(2, 0, 1)
    wbt = np.concatenate([w.T, b[:, None]], axis=1).astype(np.float32)  # [D, K+1]
    in_maps = []
    for m in range(N_CORES):
        sl = slice(m * D_SH, (m + 1) * D_SH)
        in_maps.append({
            "x": np.ascontiguousarray(xp[sl]).reshape(N_BLK, P, B, TP),
            "wb": np.ascontiguousarray(wbt[sl]).reshape(N_BLK, P, K + 1),
        })
    return in_maps


def _collect(results):
    out = np.empty((B, T, D), dtype=np.float32)
    for m in range(N_CORES):
        o = np.asarray(results[m]["out"]).reshape(D_SH, B, T)
        out[:, :, m * D_SH:(m + 1) * D_SH] = o.transpose(1, 2, 0)
    return out


def _run(in_maps, trace=False, **kwargs):
    nc = _build()
    return bass_utils.run_bass_kernel_spmd(
        nc, in_maps, core_ids=list(range(N_CORES)), trace=trace, **kwargs)


def kernel(x, w, b):
    in_maps = _prepare(x, w, b)
    try:
        res = _run(in_maps)
    except Exception:
        # Transient NRT device errors have been observed on a cold first
        # execute; one retry (fresh compile dir) clears them.
        res = _run(in_maps)
    return _collect(res.results)



# revision 2
# speedup vs baseline: 1.9186x; 1.9186x over previous
"""Causal depthwise temporal conv (K=4) on 8 TRN2 NeuronCores, bf16.

Reference semantics (for x: [B, T, D], w: [K, D], b: [D]):
    out[bt, t, d] = sum_{j=0}^{K-1} x_pad[bt, t + j, d] * w[j, d] + b[d]
where x_pad is x left-padded with K-1 zeros along time.

Strategy (vs the f32 baseline, which was DVE-bound at ~205us):
  - bf16 end-to-end: halves HBM traffic (memory regime) AND doubles DVE
    throughput (2x_1p perf mode). Numerics: max-normalized rel err ~6e-3,
    well inside the 2e-2 gate.
  - DVE 2x/4x perf modes require 4B-aligned views, so only the EVEN
    temporal shifts (0, 2) can run fast on DVE. The ODD shifts (1, 3) go
    to the otherwise-idle TensorEngine as diagonal matmuls (depthwise
    scale == diag(w) @ x) accumulating in PSUM; the DVE partial (shifts
    0+2) is injected into the same PSUM accumulation group via an
    identity matmul; ACT drains PSUM -> bf16 SBUF, folding in the bias.
  - Per-engine busy estimate per core: TensorE ~88us, DVE ~55us,
    ACT ~62us, DMA ~34MB -> ~85-100us. Roughly balanced.
  - Tensor-parallel over channels: core m owns channels [m*512,(m+1)*512);
    depthwise conv => no collectives. Host pre-transposes to channel-major
    [D_sh, B, T+K-1] (left zero-padded) and pre-builds the per-block
    diagonal weight matrices.
"""

import numpy as np
import ml_dtypes

import concourse.bacc as bacc
import concourse.mybir as mybir
from concourse.tile import TileContext
from concourse import bass_utils

BF16 = ml_dtypes.bfloat16

B = 4            # batch
T = 4096         # sequence length
D = 4096         # channels (width)
K = 4            # temporal taps
N_CORES = 8
D_SH = D // N_CORES          # 512 channels per core
P = 128                      # SBUF partitions
N_BLK = D_SH // P            # 4 channel blocks per core
TP = T + K - 1               # padded time length
NCH = 512                    # matmul chunk = one PSUM bank of fp32
HALF = 2048                  # ACT drain granularity (4 PSUM banks)


def _build(b=B, t=T, n_blk=N_BLK):
    nc = bacc.Bacc("TRN2")
    tp = t + K - 1
    f32 = mybir.dt.float32
    bf16 = mybir.dt.bfloat16
    x = nc.dram_tensor("x", [n_blk, P, b, tp], bf16, kind="ExternalInput")
    # per-partition scalars: w0, w2, bias
    ws = nc.dram_tensor("ws", [n_blk, P, 3], f32, kind="ExternalInput")
    # per-block diagonal weights: [diag(w1) | diag(w3)]
    wd = nc.dram_tensor("wd", [n_blk, P, 2 * P], bf16, kind="ExternalInput")
    ident = nc.dram_tensor("ident", [P, P], bf16, kind="ExternalInput")
    out = nc.dram_tensor("out", [n_blk, P, b, t], bf16, kind="ExternalOutput")
    mult, add = mybir.AluOpType.mult, mybir.AluOpType.add
    ident_fn = mybir.ActivationFunctionType.Identity

    with TileContext(nc) as tc:
        with tc.tile_pool(name="xp", bufs=4) as xp, \
             tc.tile_pool(name="accp", bufs=2) as accp, \
             tc.tile_pool(name="outp", bufs=3) as outp, \
             tc.tile_pool(name="wp", bufs=2) as wp, \
             tc.tile_pool(name="pp", bufs=2, space="PSUM") as pp:
            idt = wp.tile([P, P], bf16, tag="ident", bufs=1)
            nc.sync.dma_start(idt[:], ident[:, :])
            for blk in range(n_blk):
                wst = wp.tile([P, 3], f32, tag="ws")
                nc.sync.dma_start(wst[:], ws[blk])
                wdt = wp.tile([P, 2 * P], bf16, tag="wd")
                nc.sync.dma_start(wdt[:], wd[blk])
                for bb in range(b):
                    X = xp.tile([P, tp], bf16, tag="x")
                    nc.sync.dma_start(X[:], x[blk, :, bb, :])
                    # DVE: even shifts. a0 = w0*x0 (4x mode), then
                    # a2 = w2*x2 + a0 (2x_1p; both views 4B-aligned).
                    a0 = accp.tile([P, t], bf16, tag="a0")
                    nc.vector.tensor_scalar_mul(a0[:], X[:, 0:t], wst[:, 0:1])
                    a2 = accp.tile([P, t], bf16, tag="a2")
                    nc.vector.scalar_tensor_tensor(
                        a2[:], X[:, 2:2 + t], wst[:, 1:2], a0[:], mult, add)
                    osb = outp.tile([P, t], bf16, tag="o")
                    for h in range(t // HALF):
                        ps = pp.tile([P, HALF], f32, tag="ps")
                        base = h * HALF
                        # tap-major so the PE keeps each stationary across
                        # 4 chunks (fewer weight switches)
                        for n in range(HALF // NCH):
                            c0 = base + n * NCH
                            nc.tensor.matmul(
                                ps[:, n * NCH:(n + 1) * NCH], wdt[:, 0:P],
                                X[:, 1 + c0:1 + c0 + NCH],
                                start=True, stop=False)
                        for n in range(HALF // NCH):
                            c0 = base + n * NCH
                            nc.tensor.matmul(
                                ps[:, n * NCH:(n + 1) * NCH], wdt[:, P:2 * P],
                                X[:, 3 + c0:3 + c0 + NCH],
                                start=False, stop=False)
                        for n in range(HALF // NCH):
                            c0 = base + n * NCH
                            nc.tensor.matmul(
                                ps[:, n * NCH:(n + 1) * NCH], idt[:],
                                a2[:, c0:c0 + NCH],
                                start=False, stop=True)
                        nc.scalar.activation(
                            osb[:, base:base + HALF], ps[:], ident_fn,
                            bias=wst[:, 2:3], scale=1.0)
                    nc.sync.dma_start(out[blk, :, bb, :], osb[:])
    nc.compile()
    return nc


def _prepare(x, w, b):
    x = np.asarray(x, dtype=np.float32)
    w = np.asarray(w, dtype=np.float32)
    b = np.asarray(b, dtype=np.float32)
    # channel-major, left zero-padded time: [D, B, TP], bf16
    xp = np.zeros((D, B, TP), dtype=BF16)
    xp[:, :, K - 1:] = x.transpose(2, 0, 1).astype(BF16)
    wsf = np.stack([w[0], w[2], b], axis=1).astype(np.float32)  # [D, 3]
    w1 = w[1].astype(BF16)
    w3 = w[3].astype(BF16)
    eye = np.eye(P, dtype=BF16)
    in_maps = []
    for m in range(N_CORES):
        sl = slice(m * D_SH, (m + 1) * D_SH)
        wdm = np.zeros((N_BLK, P, 2 * P), dtype=BF16)
        for blk in range(N_BLK):
            ch = slice(m * D_SH + blk * P, m * D_SH + (blk + 1) * P)
            wdm[blk, :, 0:P] = np.diag(w1[ch])
            wdm[blk, :, P:2 * P] = np.diag(w3[ch])
        in_maps.append({
            "x": np.ascontiguousarray(xp[sl]).reshape(N_BLK, P, B, TP),
            "ws": np.ascontiguousarray(wsf[sl]).reshape(N_BLK, P, 3),
            "wd": wdm,
            "ident": eye,
        })
    return in_maps


def _collect(results):
    out = np.empty((B, T, D), dtype=np.float32)
    for m in range(N_CORES):
        o = np.asarray(results[m]["out"]).astype(np.float32).reshape(D_SH, B, T)
        out[:, :, m * D_SH:(m + 1) * D_SH] = o.transpose(1, 2, 0)
    return out


def _run(in_maps, trace=False, **kwargs):
    nc = _build()
    return bass_utils.run_bass_kernel_spmd(
        nc, in_maps, core_ids=list(range(N_CORES)), trace=trace, **kwargs)


def kernel(x, w, b):
    in_maps = _prepare(x, w, b)
    try:
        res = _run(in_maps)
    except Exception:
        # Transient NRT device errors have been observed on a cold first
        # execute; one retry (fresh compile dir) clears them.
        res = _run(in_maps)
    return _collect(res.results)
